# revision 46
# baseline (speedup 1.0000x reference)
"""Trainium2 Bass kernel: CausalSelfAttention (B=1, T=2048, C=4096, H=32, HS=128, NE=32).

Sharding: tensor-parallel over heads — 4 heads per core on 8 cores.

The big matmuls (QKV and output projection) run in fp8e4 DoubleRow mode
at 2 MACs/PE-cell/cycle with a hi/lo split for accuracy: every operand
X is represented as X_hi + X_lo (two fp8e4 tensors, power-of-2
pre-scaled), and X@W is computed with the three significant cross
products  Xhi@Whi + Xlo@Whi + Xhi@Wlo  (the lo·lo term is ~2^-8
relative — dropped).  Each DoubleRow instruction contracts TWO 128-deep
k-tiles (operand pairs along the free dim), so the 3-term split costs
0.75x the bf16 PE time.  The fp8 scales (x·16, W·64) are descaled at
the PSUM drain via the ACT scale parameter, or folded into the
softmax-denominator `ones` constant (64 instead of 1) and the
host-prescaled V bias, so no extra element-wise work is added.

Attention itself (S = QK^T, P@V: shallow or DVE-coupled contractions)
stays bf16 — fp8 would add instructions without cycles saved, or cost
an expensive P split.

Per core:
- QKV in one streamed pass over x (all weights resident in SBUF), the
  startup weight/x stream on a single DMA queue in exact consumption
  order; RoPE via stream_shuffle on the DVE.
- Causally-tight attention, emitted as one flat software pipeline
  across all (chunk, head) units: S-pair matmuls at step g, paired
  exp on ACT at g-1, B/Y matmuls at g-3.  The causal triangle mask is
  accumulated on the PE (ident.T @ msk matmul), the softmax
  denominator uses a 4-way DVE add tree of P tiles feeding
  ones[128,128] broadcast-matmuls.
- Per 512-query chunk, the 4 heads' y is split into fp8 hi/lo (ACT
  cast + DVE subtract, done on the local 1/8 slice BEFORE the gather
  so the collective replicates the split for free), AllGathered, and
  the core's 512-column shard of the output projection for that chunk
  is computed in fp8 DoubleRow, overlapped with the next chunk's
  attention.

Host re-assembles the full (1, 2048, 4096) output from the 8 cores'
512-row outT shards.
"""

import sys

sys.path.insert(0, "/opt/trn_rl_repo")

import numpy as np
import ml_dtypes

import concourse.bass as bass
import concourse.bacc as bacc
import concourse.mybir as mybir
from concourse import tile
from concourse.bass_utils import run_bass_kernel_spmd

BF16 = mybir.dt.bfloat16
F32 = mybir.dt.float32
FP8 = mybir.dt.float8e4
DR = mybir.MatmulPerfMode.DoubleRow
NP8 = ml_dtypes.float8_e4m3

B, T, C = 1, 2048, 4096
H, HS, NE = 32, 128, 128 // 4  # 32 heads, head_size 128, rope dims 32
NCORES = 8
HL = H // NCORES  # 4 local heads per core
SCALE = 1.0 / float(np.sqrt(HS))
MASK_VAL = -900.0  # pre-scale additive mask; exp((s-900)*scale) ~ 3e-35 -> harmless

SX = 16.0  # fp8 pre-scale for activations (x, y)
SW = 64.0  # fp8 pre-scale for weights
DESCALE = 1.0 / (SX * SW)  # applied at PSUM drains
ONES_VAL = SX * SW / SX  # 64: bps=64*sum(p) so ys*rcp = 16*y (fp8-ready)

QKT = 256  # QKV pass t-chunk
NQC = T // QKT  # 8 qkv chunks
NCH = T // 512  # 4 attention/proj chunks of 512 queries
NT = T // 128  # 16 k-tiles
NPAIR = 16  # 32 k-tiles of C contracted as 16 DoubleRow pairs
ROT32 = list(range(16, 32)) + list(range(16))  # rope partition rotation


def _build_program(repeat=1, collective=True):
    nc = bacc.Bacc(
        "TRN2",
        target_bir_lowering=False,
        debug=False,
        num_devices=NCORES if collective else 1,
    )

    # x hi/lo packed along the innermost dim per 256-query chunk so every
    # DMA line is 512B ([hi 256B | lo 256B] per (row, chunk))
    x2 = nc.dram_tensor("x2", [C, NQC, 2 * QKT], FP8, kind="ExternalInput")
    wqk_hiT = nc.dram_tensor("wqk_hiT", [C, 2 * HL * 128], FP8, kind="ExternalInput")
    wqk_loT = nc.dram_tensor("wqk_loT", [C, 2 * HL * 128], FP8, kind="ExternalInput")
    wv_hiT = nc.dram_tensor("wv_hiT", [C, HL * 128], FP8, kind="ExternalInput")
    wv_loT = nc.dram_tensor("wv_loT", [C, HL * 128], FP8, kind="ExternalInput")
    wp_hiT = nc.dram_tensor("wp_hiT", [C, 512], FP8, kind="ExternalInput")
    wp_loT = nc.dram_tensor("wp_loT", [C, 512], FP8, kind="ExternalInput")
    b_qk = nc.dram_tensor("b_qk", [128, 2 * HL], F32, kind="ExternalInput")
    b_v = nc.dram_tensor("b_v", [128, HL * 128], F32, kind="ExternalInput")
    b_p = nc.dram_tensor("b_p", [128, 4], F32, kind="ExternalInput")
    cosT = nc.dram_tensor("cosT", [NE, T], BF16, kind="ExternalInput")
    sin_pm = nc.dram_tensor("sin_pm", [NE, T], BF16, kind="ExternalInput")
    trimask = nc.dram_tensor("trimask", [128, 128], BF16, kind="ExternalInput")
    ident = nc.dram_tensor("ident", [128, 128], BF16, kind="ExternalInput")
    outT = nc.dram_tensor("outT", [512, T], F32, kind="ExternalOutput")

    # internal DRAM for the per-chunk AllGathers of y (fp8 hi|lo packed)
    y_in = nc.dram_tensor("y_in", [NCH, HL * 128, 2 * 512], FP8)
    y_outs = [
        nc.dram_tensor(f"y_out{qc}", [H * 128, 2 * 512], FP8, addr_space="Shared")
        for qc in range(NCH)
    ]

    x2_r = x2.ap().rearrange("(ct p) c t -> p ct c t", p=128)  # (128,32,NQC,512)
    wqkh_r = wqk_hiT.ap().rearrange("(ct p) r -> p ct r", p=128)  # (128, 32, 1024)
    wqkl_r = wqk_loT.ap().rearrange("(ct p) r -> p ct r", p=128)
    wvh_r = wv_hiT.ap().rearrange("(ct p) r -> p ct r", p=128)  # (128, 32, 512)
    wvl_r = wv_loT.ap().rearrange("(ct p) r -> p ct r", p=128)
    wph_r = wp_hiT.ap().rearrange("(jt p) r -> p jt r", p=128)  # (128, 32, 512)
    wpl_r = wp_loT.ap().rearrange("(jt p) r -> p jt r", p=128)
    # y_out{qc} row block b (=global head) holds head b: view as (128, 32, 1024)
    yout_rs = [y.ap().rearrange("(b p) q -> p b q", p=128) for y in y_outs]
    yin_r = y_in.ap().rearrange("c (l p) q -> c p l q", p=128)  # (NCH,128,HL,1024)

    with tile.TileContext(nc) as tc:
      for _rep in range(repeat):
        with tc.tile_pool(name="outer", bufs=1) as outer:
            # persistent tiles
            qT_sb = outer.tile([128, HL, T], BF16, tag="qT")
            kT_sb = outer.tile([128, HL, T], BF16, tag="kT")
            v_sb = outer.tile([128, NT, 512], BF16, tag="v")
            cos_sb = outer.tile([NE, T], BF16, tag="cos")
            sin_sb = outer.tile([NE, T], BF16, tag="sin")
            bqk_sb = outer.tile([128, 2 * HL], F32, tag="bqk")
            bv_sb = outer.tile([128, HL * 128], F32, tag="bv")
            bp_sb = outer.tile([128, 4], F32, tag="bp")
            msk_sb = outer.tile([128, 128], BF16, tag="msk")
            id_sb = outer.tile([128, 128], BF16, tag="id")
            ones_sb = outer.tile([128, 128], BF16, tag="ones")
            # const loads go on the gpsimd (SWDGE) queue, ordered by first use
            # (bqk/cos/sin at the first QKV drain; the rest much later)
            nc.gpsimd.dma_start(bqk_sb, b_qk.ap())
            nc.gpsimd.dma_start(cos_sb, cosT.ap())
            nc.gpsimd.dma_start(sin_sb, sin_pm.ap())
            nc.gpsimd.dma_start(bv_sb, b_v.ap())
            nc.gpsimd.dma_start(msk_sb, trimask.ap())
            nc.gpsimd.dma_start(id_sb, ident.ap())
            nc.gpsimd.dma_start(bp_sb, b_p.ap())
            nc.vector.memset(ones_sb, ONES_VAL)

            # ---------- phase 1: QKV (single pass over x) ----------
            with (
                tc.tile_pool(name="wq", bufs=1) as wqpool,
                tc.tile_pool(name="xsh", bufs=2) as xpool,
                tc.tile_pool(name="tmp", bufs=6) as tpool,
                tc.tile_pool(name="rop", bufs=1) as rpool,
            ):
                wqkh_sb = wqpool.tile([128, 32, 2 * HL * 128], FP8, tag="wqkh")
                wqkl_sb = wqpool.tile([128, 32, 2 * HL * 128], FP8, tag="wqkl")
                wvh_sb = wqpool.tile([128, 32, HL * 128], FP8, tag="wvh")
                wvl_sb = wqpool.tile([128, 32, HL * 128], FP8, tag="wvl")
                # x tiles hold [hi 256 | lo 256] per k-tile row
                xts = [xpool.tile([128, 32, 2 * QKT], FP8, tag="xt", name=f"xt{i}") for i in range(2)]
                # The whole startup stream goes on the SP queue ALONE, in exact
                # consumption order: the shared DMA device grants transfers in
                # request-arrival order, so a single queue is the only way to
                # guarantee weights/x arrive at the rate the matmuls consume.
                # 2-pair (4 k-tile) groups keep the fixed ~625ns HWDGE cost
                # per transfer well under each group's ~4.6us of DMA, and the
                # big first chunk (512 queries) makes per-group PE work
                # (~5.1us) exceed the DMA rate — a PE-bound startup.
                # the first group goes pair-by-pair — and x before wqkl — so
                # the first (Whi,xhi) matmul starts as early as possible
                _groups = [(0, 2), (2, 4), (4, 6), (6, 8)] + [
                    (8 + 4 * i, 12 + 4 * i) for i in range(6)
                ]
                for G, (g0, g1) in enumerate(_groups):
                    gs = slice(g0, g1)
                    nc.sync.dma_start(wqkh_sb[:, gs, :], wqkh_r[:, gs, :])
                    if G == 0:
                        nc.sync.dma_start(xts[0][:, gs, :], x2_r[:, gs, 0, :])
                        nc.sync.dma_start(xts[1][:, gs, :], x2_r[:, gs, 1, :])
                        nc.sync.dma_start(wqkl_sb[:, gs, :], wqkl_r[:, gs, :])
                    else:
                        nc.sync.dma_start(wqkl_sb[:, gs, :], wqkl_r[:, gs, :])
                        nc.sync.dma_start(xts[0][:, gs, :], x2_r[:, gs, 0, :])
                        nc.sync.dma_start(xts[1][:, gs, :], x2_r[:, gs, 1, :])
                # v weights stream after the whole qk stream: the big chunk's
                # v sweeps are deferred until after its qk sweep, so wv is
                # never on the critical path
                for half in range(2):
                    hs = slice(16 * half, 16 * half + 16)
                    nc.sync.dma_start(wvh_sb[:, hs, :], wvh_r[:, hs, :])
                    nc.sync.dma_start(wvl_sb[:, hs, :], wvl_r[:, hs, :])

                def qk_drains(pq_of, ts, width, rt_order=(1, 0, 3, 2, 5, 4, 7, 6)):
                    # drain+rope per q/k row tile over this chunk's query
                    # window.  The ACT drain writes ALL 128 rows straight into
                    # the q/k tile; the rope chain then reads rows 0:NE
                    # (pre-rope values) and overwrites them last — the DVE
                    # queue is in-order, so the reads always precede the
                    # final in-place write.  Rope tiles are allocated at max
                    # width and sliced so each pool tag has a single size.
                    # Odd tile first: its stop closes the shared bank's
                    # accumulation group; the even drain follows on the
                    # in-order ACT queue, so it never reads an open group
                    for rt in rt_order:
                        lh = rt // 2
                        dest = kT_sb if rt % 2 else qT_sb
                        nc.scalar.activation(
                            dest[:, lh, ts],
                            pq_of(rt),
                            mybir.ActivationFunctionType.Identity,
                            bias=bqk_sb[:, rt : rt + 1],
                            scale=DESCALE,
                        )
                        # shuffle src/dst dtypes must match: rot is bf16
                        rot = rpool.tile([NE, 512], BF16, tag="rot", name="rot")[:, 0:width]
                        nc.vector.stream_shuffle(rot, dest[0:NE, lh, ts], mask=ROT32)
                        tcos = rpool.tile([NE, 512], F32, tag="tcos", name="tcos")[:, 0:width]
                        nc.vector.tensor_tensor(
                            tcos,
                            dest[0:NE, lh, ts],
                            cos_sb[:, ts],
                            op=mybir.AluOpType.mult,
                        )
                        tsin = rpool.tile([NE, 512], F32, tag="tsin", name="tsin")[:, 0:width]
                        nc.vector.tensor_tensor(
                            tsin, rot, sin_sb[:, ts], op=mybir.AluOpType.mult
                        )
                        nc.vector.tensor_tensor(
                            dest[0:NE, lh, ts], tcos, tsin, op=mybir.AluOpType.add
                        )

                # ---- big first chunk: queries 0:512 (both x tiles), qk only;
                # each q/k row tile is a full [128,512]f32 psum bank (8 banks)
                with tc.tile_pool(name="psqk0", bufs=1, space="PSUM") as psqk0:
                    pq_pairs0 = [
                        psqk0.tile([128, 2, 512], F32, tag=f"qkb{rp}", name=f"pqb{rp}")
                        for rp in range(4)
                    ]
                    pq0 = [pq_pairs0[rt // 2][:, rt % 2, :] for rt in range(8)]
                    for g in range(NPAIR):
                        ps2 = slice(2 * g, 2 * g + 2)
                        for rt in range(8):
                            rs = slice(rt * 128, rt * 128 + 128)
                            # 3-term split; Whi stationary reused by terms 0/1
                            for ti, (wt, xo) in enumerate(
                                ((wqkh_sb, 0), (wqkh_sb, QKT), (wqkl_sb, 0))
                            ):
                                for c in range(2):
                                    nc.tensor.matmul(
                                        pq0[rt][:, c * 256 : c * 256 + 256],
                                        lhsT=wt[:, ps2, rs],
                                        rhs=xts[c][:, ps2, xo : xo + QKT],
                                        start=(g == 0 and ti == 0 and c == 0),
                                        stop=(
                                            g == NPAIR - 1 and ti == 2 and c == 1
                                        ),
                                        perf_mode=DR,
                                    )
                    # drains for the banks the deferred v sweeps will reuse
                    # (rt 0..3 = psum banks 0..3); their stops land early in
                    # pair 15, so these ACT drains overlap the sweep tail
                    qk_drains(lambda rt: pq0[rt], slice(0, 512), 512, (0, 2, 1, 3))

                with (
                    tc.tile_pool(name="psv", bufs=2, space="PSUM") as psv,
                    tc.tile_pool(name="psqk", bufs=1, space="PSUM") as psqk,
                ):
                    # ---- deferred v sweeps for the big chunk (tt 0..3) ----
                    # psv opened first: its 4 banks are the first-drained
                    # qkb0/qkb1 banks.  One sub-sweep per x tile, so xts[0]
                    # frees early for the next chunk's x DMA.
                    pvb = [
                        psv.tile([128, 512], F32, tag=f"v{tt % 2}", name=f"pvb{tt}")
                        for tt in range(4)
                    ]

                    def v_big_c(c):
                        for g in range(NPAIR):
                            ps2 = slice(2 * g, 2 * g + 2)
                            for st in range(2):
                                tt = 2 * c + st
                                xh = xts[c][:, ps2, st * 128 : st * 128 + 128]
                                xl = xts[c][
                                    :, ps2, QKT + st * 128 : QKT + st * 128 + 128
                                ]
                                # wvh terms first: wvl lands a hair later
                                for ti, (xop, wt) in enumerate(
                                    ((xh, wvh_sb), (xl, wvh_sb), (xh, wvl_sb))
                                ):
                                    for ch in range(2):
                                        cs = slice(ch * 256, ch * 256 + 256)
                                        nc.tensor.matmul(
                                            pvb[tt][:, cs],
                                            lhsT=xop,
                                            rhs=wt[:, ps2, cs],
                                            start=(g == 0 and ti == 0 and ch == 0),
                                            stop=(
                                                g == NPAIR - 1
                                                and ti == 2
                                                and ch == 1
                                            ),
                                            perf_mode=DR,
                                        )
                        for st in range(2):
                            # bv host-prescaled by 1024: v_sb = 1024*(v+bias)
                            nc.vector.tensor_tensor(
                                v_sb[:, 2 * c + st, :],
                                pvb[2 * c + st],
                                bv_sb,
                                op=mybir.AluOpType.add,
                            )

                    v_big_c(0)
                    v_big_c(1)
                    qk_drains(lambda rt: pq0[rt], slice(0, 512), 512, (5, 4, 7, 6))

                    # ---- steady chunks: 256 queries each (tt 4..15) ----
                    for ci in range(NQC - 2):
                        tch = 2 + ci
                        ts = slice(tch * QKT, tch * QKT + QKT)
                        xt = xpool.tile([128, 32, 2 * QKT], FP8, tag="xt")
                        nc.sync.dma_start(xt, x2_r[:, :, tch, :])
                        pq_pairs = [
                            psqk.tile([128, 2, QKT], F32, tag=f"qk{rp}", name=f"pq{rp}")
                            for rp in range(4)
                        ]
                        pq_tiles = [pq_pairs[rt // 2][:, rt % 2, :] for rt in range(8)]
                        pv_tiles = [
                            psv.tile([128, 512], F32, tag=f"v{st}", name=f"pv{st}")
                            for st in range(2)
                        ]
                        # q/k sweep first so their drains (and the long DVE
                        # rope chains) start mid-chunk, hiding the drain tail
                        # that gates the attention phase
                        for g in range(NPAIR):
                            ps2 = slice(2 * g, 2 * g + 2)
                            xh = xt[:, ps2, 0:QKT]
                            xl = xt[:, ps2, QKT : 2 * QKT]
                            for rt in range(8):
                                rs = slice(rt * 128, rt * 128 + 128)
                                # Two q/k tiles share a 2KB PSUM bank;
                                # start=True zeroes the WHOLE bank, so only
                                # the even tile starts — the odd tile's first
                                # write lands on pending-zero bytes
                                for ti, (wt, xop) in enumerate(
                                    ((wqkh_sb, xh), (wqkh_sb, xl), (wqkl_sb, xh))
                                ):
                                    nc.tensor.matmul(
                                        pq_tiles[rt],
                                        lhsT=wt[:, ps2, rs],
                                        rhs=xop,
                                        start=(g == 0 and rt % 2 == 0 and ti == 0),
                                        stop=(
                                            g == NPAIR - 1
                                            and rt % 2 == 1
                                            and ti == 2
                                        ),
                                        perf_mode=DR,
                                    )
                        for g in range(NPAIR):
                            ps2 = slice(2 * g, 2 * g + 2)
                            for st in range(2):
                                xh = xt[:, ps2, st * 128 : st * 128 + 128]
                                xl = xt[:, ps2, QKT + st * 128 : QKT + st * 128 + 128]
                                for ti, (xop, wt) in enumerate(
                                    ((xh, wvh_sb), (xh, wvl_sb), (xl, wvh_sb))
                                ):
                                    for ch in range(2):
                                        cs = slice(ch * 256, ch * 256 + 256)
                                        nc.tensor.matmul(
                                            pv_tiles[st][:, cs],
                                            lhsT=xop,
                                            rhs=wt[:, ps2, cs],
                                            start=(g == 0 and ti == 0 and ch == 0),
                                            stop=(
                                                g == NPAIR - 1
                                                and ti == 2
                                                and ch == 1
                                            ),
                                            perf_mode=DR,
                                        )
                        qk_drains(lambda rt, pq_tiles=pq_tiles: pq_tiles[rt], ts, QKT)
                        for st in range(2):
                            tt = 2 * tch + st
                            nc.vector.tensor_tensor(
                                v_sb[:, tt, :],
                                pv_tiles[st],
                                bv_sb,
                                op=mybir.AluOpType.add,
                            )

            # ---------- phase 2: attention + per-chunk gather + projection ----------
            with (
                tc.tile_pool(name="wp", bufs=1) as wppool,
                tc.tile_pool(name="pP", bufs=6) as pP,
                tc.tile_pool(name="pStat", bufs=2) as pStat,
                tc.tile_pool(name="yc", bufs=2) as ycpool,
                tc.tile_pool(name="yf", bufs=2) as yfpool,
                tc.tile_pool(name="osb", bufs=2) as opool,
                tc.tile_pool(name="psS", bufs=2, space="PSUM") as psS,
                tc.tile_pool(name="psP", bufs=3, space="PSUM") as psP,
                tc.tile_pool(name="psB", bufs=1, space="PSUM") as psB,
            ):
                wph_sb = wppool.tile([128, 32, 512], FP8, tag="wph")
                wpl_sb = wppool.tile([128, 32, 512], FP8, tag="wpl")
                # on the gpsimd queue: keeps both the ACT sequencer (QKV
                # drains) and the SP queue (x streaming) free
                nc.gpsimd.dma_start(wph_sb, wph_r)
                nc.gpsimd.dma_start(wpl_sb, wpl_r)

                class AttnUnit:
                    """One (query-chunk, head) attention unit, emitted in
                    pipelined stages across unit boundaries."""

                    def __init__(self, qc, lh):
                        self.qc, self.lh = qc, lh
                        self.nkt = 4 * qc + 4
                        self.npairs = self.nkt // 2
                        self.nfull = 4 * qc  # kt < nfull are full-width tiles
                        self.pair_ps = []
                        self.pair_p = []
                        self.bps = None
                        self.ys = None

                    def off_of(self, kt):
                        return max(0, (kt - 4 * self.qc) * 128)

                    def s_pair(self, p):
                        qc, lh = self.qc, self.lh
                        ps = psS.tile([128, 2, 512], F32, tag="S", name="ps")
                        self.pair_ps.append(ps)
                        for j in range(2):
                            kt = 2 * p + j
                            off = self.off_of(kt)
                            diag = kt >= self.nfull
                            nc.tensor.matmul(
                                ps[:, j, off:512],
                                lhsT=kT_sb[:, lh, kt * 128 : kt * 128 + 128],
                                rhs=qT_sb[:, lh, qc * 512 + off : qc * 512 + 512],
                                start=True,
                                stop=not diag,
                            )
                            if diag:
                                # accumulate the within-block triangular mask
                                # on the PE (ident.T @ msk = msk) into the
                                # first 128-query block of this diagonal tile
                                nc.tensor.matmul(
                                    ps[:, j, off : off + 128],
                                    lhsT=id_sb,
                                    rhs=msk_sb,
                                    start=False,
                                    stop=True,
                                )

                    def exp_pair(self, p):
                        # one ACT instruction per full pair halves the per-tile
                        # activation overhead (keeps ACT under the PE rate);
                        # mixed-width diagonal pairs split per slice so no
                        # unwritten PSUM is ever read
                        off0, off1 = self.off_of(2 * p), self.off_of(2 * p + 1)
                        pq = pP.tile([128, 2, 512], BF16, tag="P", name="pq")
                        self.pair_p.append(pq)
                        if off0 == off1:
                            nc.scalar.activation(
                                pq[:, :, off0:512],
                                self.pair_ps[p][:, :, off0:512],
                                mybir.ActivationFunctionType.Exp,
                                scale=SCALE,
                            )
                        else:
                            for j, off in ((0, off0), (1, off1)):
                                nc.scalar.activation(
                                    pq[:, j, off:512],
                                    self.pair_ps[p][:, j, off:512],
                                    mybir.ActivationFunctionType.Exp,
                                    scale=SCALE,
                                )

                    def emit_b(self, rhs_ap, last):
                        # all denominator matmuls run one by-step late (they
                        # only feed the unit-final reciprocal), so the first
                        # one never stalls on the previous unit's psB recycle
                        nc.tensor.matmul(
                            self.bps[:, 512 - rhs_ap.shape[-1] : 512],
                            lhsT=ones_sb,
                            rhs=rhs_ap,
                            start=not self.b_started,
                            stop=last,
                        )
                        self.b_started = True

                    def by_pair(self, p):
                        nkt, lh = self.nkt, self.lh
                        if p == 0:
                            self.bps = psB.tile([128, 512], F32, tag="B", name="b")
                            self.ys = psP.tile([128, 512], F32, tag="po", name="y")
                            self.padds = []
                            self.pending = []
                            self.b_started = False
                        pq = self.pair_p[p]
                        # Y matmuls first: gives the DVE time for the add tree
                        for j in range(2):
                            kt = 2 * p + j
                            off = self.off_of(kt)
                            nc.tensor.matmul(
                                self.ys[:, off:512],
                                lhsT=v_sb[:, kt, lh * 128 : lh * 128 + 128],
                                rhs=pq[:, j, off:512],
                                start=(kt == 0),
                                stop=(kt == nkt - 1),
                            )
                        if self.pending:
                            self.pending.pop(0)()
                        if 2 * p + 1 < self.nfull:
                            # full pair: sum P tiles on the DVE in a 4-way tree
                            # so one ones-matmul covers four tiles
                            padd = pP.tile([128, 512], BF16, tag="padd", name="pa")
                            nc.vector.tensor_tensor(
                                padd, pq[:, 0, :], pq[:, 1, :], op=mybir.AluOpType.add
                            )
                            self.padds.append(padd)
                            if len(self.padds) % 2 == 0:
                                padd2 = pP.tile(
                                    [128, 512], BF16, tag="padd2", name="pa2"
                                )
                                nc.vector.tensor_tensor(
                                    padd2,
                                    self.padds[-2],
                                    self.padds[-1],
                                    op=mybir.AluOpType.add,
                                )
                                self.pending.append(
                                    lambda t=padd2: self.emit_b(t, False)
                                )
                        else:
                            for j in range(2):
                                kt = 2 * p + j
                                off = self.off_of(kt)
                                self.pending.append(
                                    lambda t=pq[:, j, off:512], l=(kt == nkt - 1):
                                        self.emit_b(t, l)
                                )

                    def finish(self, yTc, yhi, ylo):
                        for f in self.pending:
                            f()
                        self.pending = []
                        lh = self.lh
                        rcp = pStat.tile([128, 512], F32, tag="rcp", name="rc")
                        nc.vector.reciprocal(rcp, self.bps)
                        # ys=1024*y_unnorm, bps=64*sum(p): yTc = 16*y
                        nc.vector.tensor_tensor(
                            yTc[:, lh, :], self.ys, rcp, op=mybir.AluOpType.mult
                        )
                        # fp8 hi/lo split of this head's slice BEFORE the
                        # gather — the collective replicates the split 8x.
                        # The cast runs on the Pool engine (right before its
                        # queue's y_in stores) so the ACT queue stays free
                        # for the exp chain.
                        nc.gpsimd.tensor_copy(yhi[:, lh, :], yTc[:, lh, :])
                        nc.vector.tensor_tensor(
                            ylo[:, lh, :],
                            yTc[:, lh, :],
                            yhi[:, lh, :],
                            op=mybir.AluOpType.subtract,
                        )
                        # stage this head's slice (shrinks the gather tail)
                        nc.gpsimd.dma_start(
                            yin_r[self.qc][:, lh, 0:512], yhi[:, lh, :]
                        )
                        nc.gpsimd.dma_start(
                            yin_r[self.qc][:, lh, 512:1024], ylo[:, lh, :]
                        )

                def proj_drain(qc, ot, ps, hw=512, cl=0):
                    ob = opool.tile([128, 512], F32, tag="ob", name="ob")
                    nc.scalar.activation(
                        ob[:, 0:hw],
                        ps,
                        mybir.ActivationFunctionType.Identity,
                        bias=bp_sb[:, ot : ot + 1],
                        scale=DESCALE,
                    )
                    nc.sync.dma_start(
                        outT.ap()[
                            ot * 128 : ot * 128 + 128,
                            qc * 512 + cl : qc * 512 + cl + hw,
                        ],
                        ob[:, 0:hw],
                    )

                def proj_mm(ps, yf, bs, ot, ch, start, stop, out_ch=None):
                    oc = ch if out_ch is None else out_ch
                    yh = yf[:, bs, ch * 256 : ch * 256 + 256]
                    yl = yf[:, bs, 512 + ch * 256 : 512 + ch * 256 + 256]
                    for ti, (wt, yop) in enumerate(
                        ((wph_sb, yh), (wph_sb, yl), (wpl_sb, yh))
                    ):
                        nc.tensor.matmul(
                            ps[:, oc * 256 : oc * 256 + 256],
                            lhsT=wt[:, bs, ot * 128 : ot * 128 + 128],
                            rhs=yop,
                            start=(start and ti == 0),
                            stop=(stop and ti == 2),
                            perf_mode=DR,
                        )

                def proj_chunk(qc, yf):
                    for ot in range(4):
                        ps = psP.tile([128, 512], F32, tag="po", name="po")
                        for g in range(NPAIR):
                            bs = slice(2 * g, 2 * g + 2)
                            for ch in range(2):
                                proj_mm(
                                    ps, yf, bs, ot, ch,
                                    start=(g == 0 and ch == 0),
                                    stop=(g == NPAIR - 1 and ch == 1),
                                )
                        proj_drain(qc, ot, ps)

                def proj_chunk_final(qc, yf):
                    # final chunk, ot-major like proj_chunk, but the very
                    # last output tile runs as two column-halves in separate
                    # banks so the first half's drain+store overlaps the
                    # second half's matmuls (shorter final tail)
                    for ot in range(3):
                        ps = psP.tile([128, 512], F32, tag="po", name="po")
                        for g in range(NPAIR):
                            bs = slice(2 * g, 2 * g + 2)
                            for ch in range(2):
                                proj_mm(
                                    ps, yf, bs, ot, ch,
                                    start=(g == 0 and ch == 0),
                                    stop=(g == NPAIR - 1 and ch == 1),
                                )
                        proj_drain(qc, ot, ps)
                    for ch in range(2):
                        ps = psP.tile([128, 256], F32, tag="po", name=f"po3{ch}")
                        for g in range(NPAIR):
                            bs = slice(2 * g, 2 * g + 2)
                            proj_mm(
                                ps, yf, bs, 3, ch,
                                start=(g == 0),
                                stop=(g == NPAIR - 1),
                                out_ch=0,
                            )
                        proj_drain(qc, 3, ps, hw=256, cl=ch * 256)

                def gather_chunk(qc):
                    if collective:
                        nc.gpsimd.collective_compute(
                            "AllGather",
                            mybir.AluOpType.bypass,
                            replica_groups=[list(range(NCORES))],
                            ins=[y_in.ap()[qc].opt()],
                            outs=[y_outs[qc].ap().opt()],
                        )
                    else:
                        nc.gpsimd.dma_start(
                            y_outs[qc].ap()[0 : HL * 128, :], y_in.ap()[qc]
                        )
                    # yf loads ride the SP queue (idle in phase 2 besides the
                    # outT stores), keeping Pool free for gathers/stores
                    yf = yfpool.tile([128, 32, 2 * 512], FP8, tag="yf", name="yf")
                    for g in range(8):
                        bs = slice(4 * g, 4 * g + 4)
                        nc.sync.dma_start(yf[:, bs, :], yout_rs[qc][:, bs, :])
                    yfs[qc] = yf

                # flat software pipeline across all units: s_pair(step g),
                # exp(g-1), B/Y(g-2) — each unit's first exp is processed
                # while the previous unit's B/Y matmuls keep the PE busy.
                # Chunk order [3, 0, 1, 2]: the big qc3 units provide dense
                # PE work while the pipeline fills, and the thin qc0/qc1
                # steps (which otherwise stall on ACT/DVE latency) are
                # interleaved with proj matmuls of already-gathered chunks.
                qc_order = [3, 0, 1, 2]
                steps = []
                post = {}  # step idx -> (block idx, qc) to gather after
                for bi, qc in enumerate(qc_order):
                    for lh in range(HL):
                        u = AttnUnit(qc, lh)
                        steps.extend((u, p) for p in range(u.npairs))
                    post[len(steps) - 1] = (bi, qc)

                yfs = {}
                yTcs = {}
                for g in range(len(steps) + 3):
                    if g < len(steps):
                        steps[g][0].s_pair(steps[g][1])
                    if 0 <= g - 1 < len(steps):
                        u, p = steps[g - 1]
                        u.exp_pair(p)
                    if 0 <= g - 3 < len(steps):
                        u, p = steps[g - 3]
                        u.by_pair(p)
                        if p == u.npairs - 1:
                            if u.lh == 0:
                                yTcs[u.qc] = (
                                    ycpool.tile(
                                        [128, HL, 512], BF16, tag="yc", name="yc"
                                    ),
                                    ycpool.tile(
                                        [128, HL, 512], FP8, tag="yhi", name="yh"
                                    ),
                                    ycpool.tile(
                                        [128, HL, 512], FP8, tag="ylo", name="yl"
                                    ),
                                )
                            u.finish(*yTcs[u.qc])
                            if u.lh == HL - 1:
                                bi, qc = post[g - 3]
                                gather_chunk(qc)
                                if bi >= 1:
                                    pqc = qc_order[bi - 1]
                                    proj_chunk(pqc, yfs[pqc])
                proj_chunk_final(qc_order[-1], yfs[qc_order[-1]])

    nc.compile()
    return nc


_NC_CACHE = {}


def _get_program(repeat=1):
    if repeat not in _NC_CACHE:
        _NC_CACHE[repeat] = _build_program(repeat)
    return _NC_CACHE[repeat]


def _bf16(a):
    return np.ascontiguousarray(a).astype(ml_dtypes.bfloat16)


def _f32(a):
    return np.ascontiguousarray(a, dtype=np.float32)


def _fp8_hilo(a, scale):
    """Scaled hi/lo fp8e4 split: a*scale = hi + lo (+ ~2^-8 residual)."""
    s = np.asarray(a, np.float32) * scale
    hi = s.astype(NP8)
    lo = (s - hi.astype(np.float32)).astype(NP8)
    return np.ascontiguousarray(hi), np.ascontiguousarray(lo)


def _make_in_maps(x, cos, sin, W_attn, b_attn, W_proj, b_proj):
    x = np.asarray(x, dtype=np.float32)
    cos = np.asarray(cos, dtype=np.float32)
    sin = np.asarray(sin, dtype=np.float32)
    W_attn = np.asarray(W_attn, dtype=np.float32)
    b_attn = np.asarray(b_attn, dtype=np.float32)
    W_proj = np.asarray(W_proj, dtype=np.float32)
    b_proj = np.asarray(b_proj, dtype=np.float32)

    xT = x[0].T  # (C, T)
    xhi, xlo = _fp8_hilo(xT, SX)
    # pack [hi | lo] along the innermost dim per 256-col chunk: (C, NQC, 512)
    x2 = np.concatenate(
        [xhi.reshape(C, NQC, QKT), xlo.reshape(C, NQC, QKT)], axis=2
    )
    cosT = _bf16(cos.T)  # (NE, T)
    sinT = sin.T
    sin_pm = _bf16(np.concatenate([-sinT[: NE // 2], sinT[NE // 2 :]], axis=0))

    # within-block triangular causal mask for diagonal 128x128 blocks:
    # entry [k, j] allowed iff j >= k
    ii = np.arange(128)[:, None]
    jj = np.arange(128)[None, :]
    trimask = _bf16(np.where(jj >= ii, 0.0, MASK_VAL))
    ident = _bf16(np.eye(128))

    # W_attn rows: head h occupies rows [h*384, (h+1)*384): q(128), k(128), v(128)
    Wr = W_attn.reshape(H, 3, HS, C)
    br = b_attn.reshape(H, 3, HS)

    in_maps = []
    for c in range(NCORES):
        hs = list(range(HL * c, HL * (c + 1)))
        # q/k rows ordered [q_h0, k_h0, q_h1, k_h1, ...] per local head
        wqk = np.concatenate(
            [Wr[h, j] for h in hs for j in (0, 1)], axis=0
        )  # (1024, C)
        wv = np.concatenate([Wr[h, 2] for h in hs], axis=0)  # (512, C)
        bqk = np.stack(
            [br[h, j] for h in hs for j in (0, 1)], axis=1
        )  # (128, 8) per-partition per row tile
        bv = np.concatenate([br[h, 2] for h in hs], axis=0)  # (512,)
        wp = W_proj[512 * c : 512 * (c + 1), :]  # (512, C)
        bp = b_proj[512 * c : 512 * (c + 1)].reshape(4, 128).T  # (128, 4)
        wqk_hi, wqk_lo = _fp8_hilo(wqk.T, SW)
        wv_hi, wv_lo = _fp8_hilo(wv.T, SW)
        wp_hi, wp_lo = _fp8_hilo(wp.T, SW)
        in_maps.append(
            {
                "x2": x2,
                "wqk_hiT": wqk_hi,
                "wqk_loT": wqk_lo,
                "wv_hiT": wv_hi,
                "wv_loT": wv_lo,
                "wp_hiT": wp_hi,
                "wp_loT": wp_lo,
                "b_qk": _f32(bqk),
                # v_sb accumulates 1024*v; fold the descale into the bias
                "b_v": _f32(np.tile((bv * SX * SW)[None, :], (128, 1))),
                "b_p": _f32(bp),
                "cosT": cosT,
                "sin_pm": sin_pm,
                "trimask": trimask,
                "ident": ident,
            }
        )
    return in_maps


def kernel(**inputs):
    in_maps = _make_in_maps(**inputs)
    nc = _get_program()
    res = run_bass_kernel_spmd(nc, in_maps, core_ids=list(range(NCORES)))
    shards = [np.asarray(res.results[c]["outT"]) for c in range(NCORES)]
    out = np.concatenate(shards, axis=0)  # (4096, 2048) = (C_out, T)
    return np.ascontiguousarray(out.T)[None].astype(np.float32)  # (1, T, C)


# revision 47
# speedup vs baseline: 1.0112x; 1.0112x over previous
"""Trainium2 Bass kernel: CausalSelfAttention (B=1, T=2048, C=4096, H=32, HS=128, NE=32).

Sharding: tensor-parallel over heads — 4 heads per core on 8 cores.

The big matmuls (QKV and output projection) run in fp8e4 DoubleRow mode
at 2 MACs/PE-cell/cycle with a hi/lo split for accuracy: every operand
X is represented as X_hi + X_lo (two fp8e4 tensors, power-of-2
pre-scaled), and X@W is computed with the three significant cross
products  Xhi@Whi + Xlo@Whi + Xhi@Wlo  (the lo·lo term is ~2^-8
relative — dropped).  Each DoubleRow instruction contracts TWO 128-deep
k-tiles (operand pairs along the free dim), so the 3-term split costs
0.75x the bf16 PE time.  The fp8 scales (x·16, W·64) are descaled at
the PSUM drain via the ACT scale parameter, or folded into the
softmax-denominator `ones` constant (64 instead of 1) and the
host-prescaled V bias, so no extra element-wise work is added.

Attention itself (S = QK^T, P@V: shallow or DVE-coupled contractions)
stays bf16 — fp8 would add instructions without cycles saved, or cost
an expensive P split.

Per core:
- QKV in one streamed pass over x (all weights resident in SBUF), the
  startup weight/x stream on a single DMA queue in exact consumption
  order; RoPE via stream_shuffle on the DVE.
- Causally-tight attention, emitted as one flat software pipeline
  across all (chunk, head) units: S-pair matmuls at step g, paired
  exp on ACT at g-1, B/Y matmuls at g-3.  The causal triangle mask is
  accumulated on the PE (ident.T @ msk matmul), the softmax
  denominator uses a 4-way DVE add tree of P tiles feeding
  ones[128,128] broadcast-matmuls.
- Per 512-query chunk, the 4 heads' y is split into fp8 hi/lo (ACT
  cast + DVE subtract, done on the local 1/8 slice BEFORE the gather
  so the collective replicates the split for free), AllGathered, and
  the core's 512-column shard of the output projection for that chunk
  is computed in fp8 DoubleRow, overlapped with the next chunk's
  attention.

Host re-assembles the full (1, 2048, 4096) output from the 8 cores'
512-row outT shards.
"""

import sys

sys.path.insert(0, "/opt/trn_rl_repo")

import numpy as np
import ml_dtypes

import concourse.bass as bass
import concourse.bacc as bacc
import concourse.mybir as mybir
from concourse import tile
from concourse.bass_utils import run_bass_kernel_spmd

BF16 = mybir.dt.bfloat16
F32 = mybir.dt.float32
FP8 = mybir.dt.float8e4
DR = mybir.MatmulPerfMode.DoubleRow
NP8 = ml_dtypes.float8_e4m3

B, T, C = 1, 2048, 4096
H, HS, NE = 32, 128, 128 // 4  # 32 heads, head_size 128, rope dims 32
NCORES = 8
HL = H // NCORES  # 4 local heads per core
SCALE = 1.0 / float(np.sqrt(HS))
MASK_VAL = -900.0  # pre-scale additive mask; exp((s-900)*scale) ~ 3e-35 -> harmless

SX = 16.0  # fp8 pre-scale for activations (x, y)
SW = 64.0  # fp8 pre-scale for weights
DESCALE = 1.0 / (SX * SW)  # applied at PSUM drains
ONES_VAL = SX * SW / SX  # 64: bps=64*sum(p) so ys*rcp = 16*y (fp8-ready)

QKT = 256  # QKV pass t-chunk
NQC = T // QKT  # 8 qkv chunks
NCH = T // 512  # 4 attention/proj chunks of 512 queries
NT = T // 128  # 16 k-tiles
NPAIR = 16  # 32 k-tiles of C contracted as 16 DoubleRow pairs
ROT32 = list(range(16, 32)) + list(range(16))  # rope partition rotation


def _build_program(repeat=1, collective=True):
    nc = bacc.Bacc(
        "TRN2",
        target_bir_lowering=False,
        debug=False,
        num_devices=NCORES if collective else 1,
    )

    # x hi/lo packed along the innermost dim per 256-query chunk so every
    # DMA line is 512B ([hi 256B | lo 256B] per (row, chunk))
    x2 = nc.dram_tensor("x2", [C, NQC, 2 * QKT], FP8, kind="ExternalInput")
    wqk_hiT = nc.dram_tensor("wqk_hiT", [C, 2 * HL * 128], FP8, kind="ExternalInput")
    wqk_loT = nc.dram_tensor("wqk_loT", [C, 2 * HL * 128], FP8, kind="ExternalInput")
    wv_hiT = nc.dram_tensor("wv_hiT", [C, HL * 128], FP8, kind="ExternalInput")
    wv_loT = nc.dram_tensor("wv_loT", [C, HL * 128], FP8, kind="ExternalInput")
    wp_hiT = nc.dram_tensor("wp_hiT", [C, 512], FP8, kind="ExternalInput")
    wp_loT = nc.dram_tensor("wp_loT", [C, 512], FP8, kind="ExternalInput")
    b_qk = nc.dram_tensor("b_qk", [128, 2 * HL], F32, kind="ExternalInput")
    b_v = nc.dram_tensor("b_v", [128, HL * 128], F32, kind="ExternalInput")
    b_p = nc.dram_tensor("b_p", [128, 4], F32, kind="ExternalInput")
    cosT = nc.dram_tensor("cosT", [NE, T], BF16, kind="ExternalInput")
    sin_pm = nc.dram_tensor("sin_pm", [NE, T], BF16, kind="ExternalInput")
    trimask = nc.dram_tensor("trimask", [128, 128], BF16, kind="ExternalInput")
    ident = nc.dram_tensor("ident", [128, 128], BF16, kind="ExternalInput")
    outT = nc.dram_tensor("outT", [512, T], F32, kind="ExternalOutput")

    # internal DRAM for the per-chunk AllGathers of y (fp8 hi|lo packed)
    y_in = nc.dram_tensor("y_in", [NCH, HL * 128, 2 * 512], FP8)
    y_outs = [
        nc.dram_tensor(f"y_out{qc}", [H * 128, 2 * 512], FP8, addr_space="Shared")
        for qc in range(NCH)
    ]

    x2_r = x2.ap().rearrange("(ct p) c t -> p ct c t", p=128)  # (128,32,NQC,512)
    wqkh_r = wqk_hiT.ap().rearrange("(ct p) r -> p ct r", p=128)  # (128, 32, 1024)
    wqkl_r = wqk_loT.ap().rearrange("(ct p) r -> p ct r", p=128)
    wvh_r = wv_hiT.ap().rearrange("(ct p) r -> p ct r", p=128)  # (128, 32, 512)
    wvl_r = wv_loT.ap().rearrange("(ct p) r -> p ct r", p=128)
    wph_r = wp_hiT.ap().rearrange("(jt p) r -> p jt r", p=128)  # (128, 32, 512)
    wpl_r = wp_loT.ap().rearrange("(jt p) r -> p jt r", p=128)
    # y_out{qc} row block b (=global head) holds head b: view as (128, 32, 1024)
    yout_rs = [y.ap().rearrange("(b p) q -> p b q", p=128) for y in y_outs]
    yin_r = y_in.ap().rearrange("c (l p) q -> c p l q", p=128)  # (NCH,128,HL,1024)

    with tile.TileContext(nc) as tc:
      for _rep in range(repeat):
        with tc.tile_pool(name="outer", bufs=1) as outer:
            # persistent tiles
            qT_sb = outer.tile([128, HL, T], BF16, tag="qT")
            kT_sb = outer.tile([128, HL, T], BF16, tag="kT")
            v_sb = outer.tile([128, NT, 512], BF16, tag="v")
            cos_sb = outer.tile([NE, T], BF16, tag="cos")
            sin_sb = outer.tile([NE, T], BF16, tag="sin")
            bqk_sb = outer.tile([128, 2 * HL], F32, tag="bqk")
            bv_sb = outer.tile([128, HL * 128], F32, tag="bv")
            bp_sb = outer.tile([128, 4], F32, tag="bp")
            msk_sb = outer.tile([128, 128], BF16, tag="msk")
            id_sb = outer.tile([128, 128], BF16, tag="id")
            ones_sb = outer.tile([128, 128], BF16, tag="ones")
            # const loads go on the gpsimd (SWDGE) queue, ordered by first use
            # (bqk/cos/sin at the first QKV drain; the rest much later)
            nc.gpsimd.dma_start(bqk_sb, b_qk.ap())
            nc.gpsimd.dma_start(cos_sb, cosT.ap())
            nc.gpsimd.dma_start(sin_sb, sin_pm.ap())
            nc.gpsimd.dma_start(bv_sb, b_v.ap())
            nc.gpsimd.dma_start(msk_sb, trimask.ap())
            nc.gpsimd.dma_start(id_sb, ident.ap())
            nc.gpsimd.dma_start(bp_sb, b_p.ap())
            nc.vector.memset(ones_sb, ONES_VAL)

            # ---------- phase 1: QKV (single pass over x) ----------
            with (
                tc.tile_pool(name="wq", bufs=1) as wqpool,
                tc.tile_pool(name="xsh", bufs=2) as xpool,
                tc.tile_pool(name="tmp", bufs=6) as tpool,
                tc.tile_pool(name="rop", bufs=1) as rpool,
            ):
                wqkh_sb = wqpool.tile([128, 32, 2 * HL * 128], FP8, tag="wqkh")
                wqkl_sb = wqpool.tile([128, 32, 2 * HL * 128], FP8, tag="wqkl")
                wvh_sb = wqpool.tile([128, 32, HL * 128], FP8, tag="wvh")
                wvl_sb = wqpool.tile([128, 32, HL * 128], FP8, tag="wvl")
                # x tiles hold [hi 256 | lo 256] per k-tile row
                xts = [xpool.tile([128, 32, 2 * QKT], FP8, tag="xt", name=f"xt{i}") for i in range(2)]
                # The whole startup stream goes on the SP queue ALONE, in exact
                # consumption order: the shared DMA device grants transfers in
                # request-arrival order, so a single queue is the only way to
                # guarantee weights/x arrive at the rate the matmuls consume.
                # 2-pair (4 k-tile) groups keep the fixed ~625ns HWDGE cost
                # per transfer well under each group's ~4.6us of DMA, and the
                # big first chunk (512 queries) makes per-group PE work
                # (~5.1us) exceed the DMA rate — a PE-bound startup.
                # the first group goes pair-by-pair — and x before wqkl — so
                # the first (Whi,xhi) matmul starts as early as possible
                _groups = [(0, 2), (2, 4), (4, 6), (6, 8)] + [
                    (8 + 4 * i, 12 + 4 * i) for i in range(6)
                ]
                for G, (g0, g1) in enumerate(_groups):
                    gs = slice(g0, g1)
                    nc.sync.dma_start(wqkh_sb[:, gs, :], wqkh_r[:, gs, :])
                    if G == 0:
                        nc.sync.dma_start(xts[0][:, gs, :], x2_r[:, gs, 0, :])
                        nc.sync.dma_start(xts[1][:, gs, :], x2_r[:, gs, 1, :])
                        nc.sync.dma_start(wqkl_sb[:, gs, :], wqkl_r[:, gs, :])
                    else:
                        nc.sync.dma_start(wqkl_sb[:, gs, :], wqkl_r[:, gs, :])
                        nc.sync.dma_start(xts[0][:, gs, :], x2_r[:, gs, 0, :])
                        nc.sync.dma_start(xts[1][:, gs, :], x2_r[:, gs, 1, :])
                # v weights stream after the whole qk stream: the big chunk's
                # v sweeps are deferred until after its qk sweep, so wv is
                # never on the critical path
                for half in range(2):
                    hs = slice(16 * half, 16 * half + 16)
                    nc.sync.dma_start(wvh_sb[:, hs, :], wvh_r[:, hs, :])
                    nc.sync.dma_start(wvl_sb[:, hs, :], wvl_r[:, hs, :])

                def qk_drains(pq_of, ts, width, rt_order=(1, 0, 3, 2, 5, 4, 7, 6)):
                    # drain+rope per q/k row tile over this chunk's query
                    # window.  The ACT drain writes ALL 128 rows straight into
                    # the q/k tile; the rope chain then reads rows 0:NE
                    # (pre-rope values) and overwrites them last — the DVE
                    # queue is in-order, so the reads always precede the
                    # final in-place write.  Rope tiles are allocated at max
                    # width and sliced so each pool tag has a single size.
                    # Odd tile first: its stop closes the shared bank's
                    # accumulation group; the even drain follows on the
                    # in-order ACT queue, so it never reads an open group
                    for rt in rt_order:
                        lh = rt // 2
                        dest = kT_sb if rt % 2 else qT_sb
                        nc.scalar.activation(
                            dest[:, lh, ts],
                            pq_of(rt),
                            mybir.ActivationFunctionType.Identity,
                            bias=bqk_sb[:, rt : rt + 1],
                            scale=DESCALE,
                        )
                        # shuffle src/dst dtypes must match: rot is bf16
                        rot = rpool.tile([NE, 512], BF16, tag="rot", name="rot")[:, 0:width]
                        nc.vector.stream_shuffle(rot, dest[0:NE, lh, ts], mask=ROT32)
                        tcos = rpool.tile([NE, 512], F32, tag="tcos", name="tcos")[:, 0:width]
                        nc.vector.tensor_tensor(
                            tcos,
                            dest[0:NE, lh, ts],
                            cos_sb[:, ts],
                            op=mybir.AluOpType.mult,
                        )
                        tsin = rpool.tile([NE, 512], F32, tag="tsin", name="tsin")[:, 0:width]
                        nc.vector.tensor_tensor(
                            tsin, rot, sin_sb[:, ts], op=mybir.AluOpType.mult
                        )
                        nc.vector.tensor_tensor(
                            dest[0:NE, lh, ts], tcos, tsin, op=mybir.AluOpType.add
                        )

                # ---- big first chunk: queries 0:512 (both x tiles), qk only;
                # each q/k row tile is a full [128,512]f32 psum bank (8 banks)
                with tc.tile_pool(name="psqk0", bufs=1, space="PSUM") as psqk0:
                    pq_pairs0 = [
                        psqk0.tile([128, 2, 512], F32, tag=f"qkb{rp}", name=f"pqb{rp}")
                        for rp in range(4)
                    ]
                    pq0 = [pq_pairs0[rt // 2][:, rt % 2, :] for rt in range(8)]
                    for g in range(NPAIR):
                        ps2 = slice(2 * g, 2 * g + 2)
                        for rt in range(8):
                            rs = slice(rt * 128, rt * 128 + 128)
                            # 3-term split; Whi stationary reused by terms 0/1
                            for ti, (wt, xo) in enumerate(
                                ((wqkh_sb, 0), (wqkh_sb, QKT), (wqkl_sb, 0))
                            ):
                                for c in range(2):
                                    nc.tensor.matmul(
                                        pq0[rt][:, c * 256 : c * 256 + 256],
                                        lhsT=wt[:, ps2, rs],
                                        rhs=xts[c][:, ps2, xo : xo + QKT],
                                        start=(g == 0 and ti == 0 and c == 0),
                                        stop=(
                                            g == NPAIR - 1 and ti == 2 and c == 1
                                        ),
                                        perf_mode=DR,
                                    )
                    # drains for the banks the deferred v sweeps will reuse
                    # (rt 0..3 = psum banks 0..3); their stops land early in
                    # pair 15, so these ACT drains overlap the sweep tail
                    qk_drains(lambda rt: pq0[rt], slice(0, 512), 512, (0, 2, 1, 3))

                with (
                    tc.tile_pool(name="psv", bufs=2, space="PSUM") as psv,
                    tc.tile_pool(name="psqk", bufs=1, space="PSUM") as psqk,
                ):
                    # ---- deferred v sweeps for the big chunk (tt 0..3) ----
                    # psv opened first: its 4 banks are the first-drained
                    # qkb0/qkb1 banks.  One sub-sweep per x tile, so xts[0]
                    # frees early for the next chunk's x DMA.
                    pvb = [
                        psv.tile([128, 512], F32, tag=f"v{tt % 2}", name=f"pvb{tt}")
                        for tt in range(4)
                    ]

                    def v_big_c(c):
                        for g in range(NPAIR):
                            ps2 = slice(2 * g, 2 * g + 2)
                            for st in range(2):
                                tt = 2 * c + st
                                xh = xts[c][:, ps2, st * 128 : st * 128 + 128]
                                xl = xts[c][
                                    :, ps2, QKT + st * 128 : QKT + st * 128 + 128
                                ]
                                # wvh terms first: wvl lands a hair later
                                for ti, (xop, wt) in enumerate(
                                    ((xh, wvh_sb), (xl, wvh_sb), (xh, wvl_sb))
                                ):
                                    for ch in range(2):
                                        cs = slice(ch * 256, ch * 256 + 256)
                                        nc.tensor.matmul(
                                            pvb[tt][:, cs],
                                            lhsT=xop,
                                            rhs=wt[:, ps2, cs],
                                            start=(g == 0 and ti == 0 and ch == 0),
                                            stop=(
                                                g == NPAIR - 1
                                                and ti == 2
                                                and ch == 1
                                            ),
                                            perf_mode=DR,
                                        )
                        for st in range(2):
                            # bv host-prescaled by 1024: v_sb = 1024*(v+bias)
                            nc.vector.tensor_tensor(
                                v_sb[:, 2 * c + st, :],
                                pvb[2 * c + st],
                                bv_sb,
                                op=mybir.AluOpType.add,
                            )

                    v_big_c(0)
                    v_big_c(1)
                    qk_drains(lambda rt: pq0[rt], slice(0, 512), 512, (5, 4, 7, 6))

                    # ---- steady chunks: 256 queries each (tt 4..15) ----
                    for ci in range(NQC - 2):
                        tch = 2 + ci
                        ts = slice(tch * QKT, tch * QKT + QKT)
                        xt = xpool.tile([128, 32, 2 * QKT], FP8, tag="xt")
                        nc.sync.dma_start(xt, x2_r[:, :, tch, :])
                        pq_pairs = [
                            psqk.tile([128, 2, QKT], F32, tag=f"qk{rp}", name=f"pq{rp}")
                            for rp in range(4)
                        ]
                        pq_tiles = [pq_pairs[rt // 2][:, rt % 2, :] for rt in range(8)]
                        pv_tiles = [
                            psv.tile([128, 512], F32, tag=f"v{st}", name=f"pv{st}")
                            for st in range(2)
                        ]
                        # q/k sweep first so their drains (and the long DVE
                        # rope chains) start mid-chunk, hiding the drain tail
                        # that gates the attention phase
                        for g in range(NPAIR):
                            ps2 = slice(2 * g, 2 * g + 2)
                            xh = xt[:, ps2, 0:QKT]
                            xl = xt[:, ps2, QKT : 2 * QKT]
                            for rt in range(8):
                                rs = slice(rt * 128, rt * 128 + 128)
                                # Two q/k tiles share a 2KB PSUM bank;
                                # start=True zeroes the WHOLE bank, so only
                                # the even tile starts — the odd tile's first
                                # write lands on pending-zero bytes
                                for ti, (wt, xop) in enumerate(
                                    ((wqkh_sb, xh), (wqkh_sb, xl), (wqkl_sb, xh))
                                ):
                                    nc.tensor.matmul(
                                        pq_tiles[rt],
                                        lhsT=wt[:, ps2, rs],
                                        rhs=xop,
                                        start=(g == 0 and rt % 2 == 0 and ti == 0),
                                        stop=(
                                            g == NPAIR - 1
                                            and rt % 2 == 1
                                            and ti == 2
                                        ),
                                        perf_mode=DR,
                                    )
                        for g in range(NPAIR):
                            ps2 = slice(2 * g, 2 * g + 2)
                            for st in range(2):
                                xh = xt[:, ps2, st * 128 : st * 128 + 128]
                                xl = xt[:, ps2, QKT + st * 128 : QKT + st * 128 + 128]
                                for ti, (xop, wt) in enumerate(
                                    ((xh, wvh_sb), (xh, wvl_sb), (xl, wvh_sb))
                                ):
                                    for ch in range(2):
                                        cs = slice(ch * 256, ch * 256 + 256)
                                        nc.tensor.matmul(
                                            pv_tiles[st][:, cs],
                                            lhsT=xop,
                                            rhs=wt[:, ps2, cs],
                                            start=(g == 0 and ti == 0 and ch == 0),
                                            stop=(
                                                g == NPAIR - 1
                                                and ti == 2
                                                and ch == 1
                                            ),
                                            perf_mode=DR,
                                        )
                        qk_drains(lambda rt, pq_tiles=pq_tiles: pq_tiles[rt], ts, QKT)
                        for st in range(2):
                            tt = 2 * tch + st
                            nc.vector.tensor_tensor(
                                v_sb[:, tt, :],
                                pv_tiles[st],
                                bv_sb,
                                op=mybir.AluOpType.add,
                            )

            # ---------- phase 2: attention + per-chunk gather + projection ----------
            with (
                tc.tile_pool(name="wp", bufs=1) as wppool,
                tc.tile_pool(name="pP", bufs=6) as pP,
                tc.tile_pool(name="pStat", bufs=2) as pStat,
                tc.tile_pool(name="yc", bufs=2) as ycpool,
                tc.tile_pool(name="yf", bufs=2) as yfpool,
                tc.tile_pool(name="osb", bufs=2) as opool,
                tc.tile_pool(name="psS", bufs=2, space="PSUM") as psS,
                tc.tile_pool(name="psP", bufs=3, space="PSUM") as psP,
                tc.tile_pool(name="psB", bufs=1, space="PSUM") as psB,
            ):
                wph_sb = wppool.tile([128, 32, 512], FP8, tag="wph")
                wpl_sb = wppool.tile([128, 32, 512], FP8, tag="wpl")
                # on the gpsimd queue: keeps both the ACT sequencer (QKV
                # drains) and the SP queue (x streaming) free
                nc.gpsimd.dma_start(wph_sb, wph_r)
                nc.gpsimd.dma_start(wpl_sb, wpl_r)

                class AttnUnit:
                    """One (query-chunk, head) attention unit, emitted in
                    pipelined stages across unit boundaries."""

                    def __init__(self, qc, lh):
                        self.qc, self.lh = qc, lh
                        self.nkt = 4 * qc + 4
                        self.npairs = self.nkt // 2
                        self.nfull = 4 * qc  # kt < nfull are full-width tiles
                        self.pair_ps = []
                        self.pair_p = []
                        self.bps = None
                        self.ys = None

                    def off_of(self, kt):
                        return max(0, (kt - 4 * self.qc) * 128)

                    def s_pair(self, p):
                        qc, lh = self.qc, self.lh
                        ps = psS.tile([128, 2, 512], F32, tag="S", name="ps")
                        self.pair_ps.append(ps)
                        for j in range(2):
                            kt = 2 * p + j
                            off = self.off_of(kt)
                            diag = kt >= self.nfull
                            nc.tensor.matmul(
                                ps[:, j, off:512],
                                lhsT=kT_sb[:, lh, kt * 128 : kt * 128 + 128],
                                rhs=qT_sb[:, lh, qc * 512 + off : qc * 512 + 512],
                                start=True,
                                stop=not diag,
                            )
                            if diag:
                                # accumulate the within-block triangular mask
                                # on the PE (ident.T @ msk = msk) into the
                                # first 128-query block of this diagonal tile
                                nc.tensor.matmul(
                                    ps[:, j, off : off + 128],
                                    lhsT=id_sb,
                                    rhs=msk_sb,
                                    start=False,
                                    stop=True,
                                )

                    def exp_pair(self, p):
                        # ONE ACT instruction per pair, even for mixed-width
                        # diagonal pairs: start=True zero-filled the whole
                        # psum bank, so the unwritten [off0:off1) slice of the
                        # second tile reads 0 -> exp(0)=1, which lands in a
                        # region no Y/B consumer ever touches.  Halving the
                        # instruction count shortens the latency-critical
                        # S->exp->BY chain in the thin units.
                        off0 = self.off_of(2 * p)
                        pq = pP.tile([128, 2, 512], BF16, tag="P", name="pq")
                        self.pair_p.append(pq)
                        nc.scalar.activation(
                            pq[:, :, off0:512],
                            self.pair_ps[p][:, :, off0:512],
                            mybir.ActivationFunctionType.Exp,
                            scale=SCALE,
                        )

                    def emit_b(self, rhs_ap, last):
                        # all denominator matmuls run one by-step late (they
                        # only feed the unit-final reciprocal), so the first
                        # one never stalls on the previous unit's psB recycle
                        nc.tensor.matmul(
                            self.bps[:, 512 - rhs_ap.shape[-1] : 512],
                            lhsT=ones_sb,
                            rhs=rhs_ap,
                            start=not self.b_started,
                            stop=last,
                        )
                        self.b_started = True

                    def by_pair(self, p):
                        nkt, lh = self.nkt, self.lh
                        if p == 0:
                            self.bps = psB.tile([128, 512], F32, tag="B", name="b")
                            self.ys = psP.tile([128, 512], F32, tag="po", name="y")
                            self.padds = []
                            self.pending = []
                            self.b_started = False
                        pq = self.pair_p[p]
                        # Y matmuls first: gives the DVE time for the add tree
                        for j in range(2):
                            kt = 2 * p + j
                            off = self.off_of(kt)
                            nc.tensor.matmul(
                                self.ys[:, off:512],
                                lhsT=v_sb[:, kt, lh * 128 : lh * 128 + 128],
                                rhs=pq[:, j, off:512],
                                start=(kt == 0),
                                stop=(kt == nkt - 1),
                            )
                        if self.pending:
                            self.pending.pop(0)()
                        if 2 * p + 1 < self.nfull:
                            # full pair: sum P tiles on the DVE in a 4-way tree
                            # so one ones-matmul covers four tiles
                            padd = pP.tile([128, 512], BF16, tag="padd", name="pa")
                            nc.vector.tensor_tensor(
                                padd, pq[:, 0, :], pq[:, 1, :], op=mybir.AluOpType.add
                            )
                            self.padds.append(padd)
                            if len(self.padds) % 2 == 0:
                                padd2 = pP.tile(
                                    [128, 512], BF16, tag="padd2", name="pa2"
                                )
                                nc.vector.tensor_tensor(
                                    padd2,
                                    self.padds[-2],
                                    self.padds[-1],
                                    op=mybir.AluOpType.add,
                                )
                                self.pending.append(
                                    lambda t=padd2: self.emit_b(t, False)
                                )
                        else:
                            for j in range(2):
                                kt = 2 * p + j
                                off = self.off_of(kt)
                                self.pending.append(
                                    lambda t=pq[:, j, off:512], l=(kt == nkt - 1):
                                        self.emit_b(t, l)
                                )

                    def finish(self, yTc, yhi, ylo):
                        for f in self.pending:
                            f()
                        self.pending = []
                        lh = self.lh
                        rcp = pStat.tile([128, 512], F32, tag="rcp", name="rc")
                        nc.vector.reciprocal(rcp, self.bps)
                        # ys=1024*y_unnorm, bps=64*sum(p): yTc = 16*y
                        nc.vector.tensor_tensor(
                            yTc[:, lh, :], self.ys, rcp, op=mybir.AluOpType.mult
                        )
                        # fp8 hi/lo split of this head's slice BEFORE the
                        # gather — the collective replicates the split 8x.
                        # The cast runs on the Pool engine (right before its
                        # queue's y_in stores) so the ACT queue stays free
                        # for the exp chain.
                        nc.gpsimd.tensor_copy(yhi[:, lh, :], yTc[:, lh, :])
                        nc.vector.tensor_tensor(
                            ylo[:, lh, :],
                            yTc[:, lh, :],
                            yhi[:, lh, :],
                            op=mybir.AluOpType.subtract,
                        )
                        # stage this head's slice (shrinks the gather tail)
                        nc.gpsimd.dma_start(
                            yin_r[self.qc][:, lh, 0:512], yhi[:, lh, :]
                        )
                        nc.gpsimd.dma_start(
                            yin_r[self.qc][:, lh, 512:1024], ylo[:, lh, :]
                        )

                def proj_drain(qc, ot, ps, hw=512, cl=0):
                    ob = opool.tile([128, 512], F32, tag="ob", name="ob")
                    nc.scalar.activation(
                        ob[:, 0:hw],
                        ps,
                        mybir.ActivationFunctionType.Identity,
                        bias=bp_sb[:, ot : ot + 1],
                        scale=DESCALE,
                    )
                    nc.sync.dma_start(
                        outT.ap()[
                            ot * 128 : ot * 128 + 128,
                            qc * 512 + cl : qc * 512 + cl + hw,
                        ],
                        ob[:, 0:hw],
                    )

                def proj_mm(ps, yf, bs, ot, ch, start, stop, out_ch=None):
                    oc = ch if out_ch is None else out_ch
                    yh = yf[:, bs, ch * 256 : ch * 256 + 256]
                    yl = yf[:, bs, 512 + ch * 256 : 512 + ch * 256 + 256]
                    for ti, (wt, yop) in enumerate(
                        ((wph_sb, yh), (wph_sb, yl), (wpl_sb, yh))
                    ):
                        nc.tensor.matmul(
                            ps[:, oc * 256 : oc * 256 + 256],
                            lhsT=wt[:, bs, ot * 128 : ot * 128 + 128],
                            rhs=yop,
                            start=(start and ti == 0),
                            stop=(stop and ti == 2),
                            perf_mode=DR,
                        )

                def proj_chunk(qc, yf):
                    for ot in range(4):
                        ps = psP.tile([128, 512], F32, tag="po", name="po")
                        for g in range(NPAIR):
                            bs = slice(2 * g, 2 * g + 2)
                            for ch in range(2):
                                proj_mm(
                                    ps, yf, bs, ot, ch,
                                    start=(g == 0 and ch == 0),
                                    stop=(g == NPAIR - 1 and ch == 1),
                                )
                        proj_drain(qc, ot, ps)

                def proj_chunk_final(qc, yf):
                    # final chunk, ot-major like proj_chunk, but the very
                    # last output tile runs as two column-halves in separate
                    # banks so the first half's drain+store overlaps the
                    # second half's matmuls (shorter final tail)
                    for ot in range(3):
                        ps = psP.tile([128, 512], F32, tag="po", name="po")
                        for g in range(NPAIR):
                            bs = slice(2 * g, 2 * g + 2)
                            for ch in range(2):
                                proj_mm(
                                    ps, yf, bs, ot, ch,
                                    start=(g == 0 and ch == 0),
                                    stop=(g == NPAIR - 1 and ch == 1),
                                )
                        proj_drain(qc, ot, ps)
                    for ch in range(2):
                        ps = psP.tile([128, 256], F32, tag="po", name=f"po3{ch}")
                        for g in range(NPAIR):
                            bs = slice(2 * g, 2 * g + 2)
                            proj_mm(
                                ps, yf, bs, 3, ch,
                                start=(g == 0),
                                stop=(g == NPAIR - 1),
                                out_ch=0,
                            )
                        proj_drain(qc, 3, ps, hw=256, cl=ch * 256)

                def gather_chunk(qc):
                    if collective:
                        nc.gpsimd.collective_compute(
                            "AllGather",
                            mybir.AluOpType.bypass,
                            replica_groups=[list(range(NCORES))],
                            ins=[y_in.ap()[qc].opt()],
                            outs=[y_outs[qc].ap().opt()],
                        )
                    else:
                        nc.gpsimd.dma_start(
                            y_outs[qc].ap()[0 : HL * 128, :], y_in.ap()[qc]
                        )
                    # yf loads ride the SP queue (idle in phase 2 besides the
                    # outT stores), keeping Pool free for gathers/stores
                    yf = yfpool.tile([128, 32, 2 * 512], FP8, tag="yf", name="yf")
                    for g in range(8):
                        bs = slice(4 * g, 4 * g + 4)
                        nc.sync.dma_start(yf[:, bs, :], yout_rs[qc][:, bs, :])
                    yfs[qc] = yf

                # flat software pipeline across all units: s_pair(step g),
                # exp(g-1), B/Y(g-2) — each unit's first exp is processed
                # while the previous unit's B/Y matmuls keep the PE busy.
                # Chunk order [3, 0, 1, 2]: the big qc3 units provide dense
                # PE work while the pipeline fills, and the thin qc0/qc1
                # steps (which otherwise stall on ACT/DVE latency) are
                # interleaved with proj matmuls of already-gathered chunks.
                qc_order = [3, 0, 1, 2]
                steps = []
                post = {}  # step idx -> (block idx, qc) to gather after
                for bi, qc in enumerate(qc_order):
                    for lh in range(HL):
                        u = AttnUnit(qc, lh)
                        steps.extend((u, p) for p in range(u.npairs))
                    post[len(steps) - 1] = (bi, qc)

                yfs = {}
                yTcs = {}
                for g in range(len(steps) + 3):
                    if g < len(steps):
                        steps[g][0].s_pair(steps[g][1])
                    if 0 <= g - 1 < len(steps):
                        u, p = steps[g - 1]
                        u.exp_pair(p)
                    if 0 <= g - 3 < len(steps):
                        u, p = steps[g - 3]
                        u.by_pair(p)
                        if p == u.npairs - 1:
                            if u.lh == 0:
                                yTcs[u.qc] = (
                                    ycpool.tile(
                                        [128, HL, 512], BF16, tag="yc", name="yc"
                                    ),
                                    ycpool.tile(
                                        [128, HL, 512], FP8, tag="yhi", name="yh"
                                    ),
                                    ycpool.tile(
                                        [128, HL, 512], FP8, tag="ylo", name="yl"
                                    ),
                                )
                            u.finish(*yTcs[u.qc])
                            if u.lh == HL - 1:
                                bi, qc = post[g - 3]
                                gather_chunk(qc)
                                if bi >= 1:
                                    pqc = qc_order[bi - 1]
                                    proj_chunk(pqc, yfs[pqc])
                proj_chunk_final(qc_order[-1], yfs[qc_order[-1]])

    nc.compile()
    return nc


_NC_CACHE = {}


def _get_program(repeat=1):
    if repeat not in _NC_CACHE:
        _NC_CACHE[repeat] = _build_program(repeat)
    return _NC_CACHE[repeat]


def _bf16(a):
    return np.ascontiguousarray(a).astype(ml_dtypes.bfloat16)


def _f32(a):
    return np.ascontiguousarray(a, dtype=np.float32)


def _fp8_hilo(a, scale):
    """Scaled hi/lo fp8e4 split: a*scale = hi + lo (+ ~2^-8 residual)."""
    s = np.asarray(a, np.float32) * scale
    hi = s.astype(NP8)
    lo = (s - hi.astype(np.float32)).astype(NP8)
    return np.ascontiguousarray(hi), np.ascontiguousarray(lo)


def _make_in_maps(x, cos, sin, W_attn, b_attn, W_proj, b_proj):
    x = np.asarray(x, dtype=np.float32)
    cos = np.asarray(cos, dtype=np.float32)
    sin = np.asarray(sin, dtype=np.float32)
    W_attn = np.asarray(W_attn, dtype=np.float32)
    b_attn = np.asarray(b_attn, dtype=np.float32)
    W_proj = np.asarray(W_proj, dtype=np.float32)
    b_proj = np.asarray(b_proj, dtype=np.float32)

    xT = x[0].T  # (C, T)
    xhi, xlo = _fp8_hilo(xT, SX)
    # pack [hi | lo] along the innermost dim per 256-col chunk: (C, NQC, 512)
    x2 = np.concatenate(
        [xhi.reshape(C, NQC, QKT), xlo.reshape(C, NQC, QKT)], axis=2
    )
    cosT = _bf16(cos.T)  # (NE, T)
    sinT = sin.T
    sin_pm = _bf16(np.concatenate([-sinT[: NE // 2], sinT[NE // 2 :]], axis=0))

    # within-block triangular causal mask for diagonal 128x128 blocks:
    # entry [k, j] allowed iff j >= k
    ii = np.arange(128)[:, None]
    jj = np.arange(128)[None, :]
    trimask = _bf16(np.where(jj >= ii, 0.0, MASK_VAL))
    ident = _bf16(np.eye(128))

    # W_attn rows: head h occupies rows [h*384, (h+1)*384): q(128), k(128), v(128)
    Wr = W_attn.reshape(H, 3, HS, C)
    br = b_attn.reshape(H, 3, HS)

    in_maps = []
    for c in range(NCORES):
        hs = list(range(HL * c, HL * (c + 1)))
        # q/k rows ordered [q_h0, k_h0, q_h1, k_h1, ...] per local head
        wqk = np.concatenate(
            [Wr[h, j] for h in hs for j in (0, 1)], axis=0
        )  # (1024, C)
        wv = np.concatenate([Wr[h, 2] for h in hs], axis=0)  # (512, C)
        bqk = np.stack(
            [br[h, j] for h in hs for j in (0, 1)], axis=1
        )  # (128, 8) per-partition per row tile
        bv = np.concatenate([br[h, 2] for h in hs], axis=0)  # (512,)
        wp = W_proj[512 * c : 512 * (c + 1), :]  # (512, C)
        bp = b_proj[512 * c : 512 * (c + 1)].reshape(4, 128).T  # (128, 4)
        wqk_hi, wqk_lo = _fp8_hilo(wqk.T, SW)
        wv_hi, wv_lo = _fp8_hilo(wv.T, SW)
        wp_hi, wp_lo = _fp8_hilo(wp.T, SW)
        in_maps.append(
            {
                "x2": x2,
                "wqk_hiT": wqk_hi,
                "wqk_loT": wqk_lo,
                "wv_hiT": wv_hi,
                "wv_loT": wv_lo,
                "wp_hiT": wp_hi,
                "wp_loT": wp_lo,
                "b_qk": _f32(bqk),
                # v_sb accumulates 1024*v; fold the descale into the bias
                "b_v": _f32(np.tile((bv * SX * SW)[None, :], (128, 1))),
                "b_p": _f32(bp),
                "cosT": cosT,
                "sin_pm": sin_pm,
                "trimask": trimask,
                "ident": ident,
            }
        )
    return in_maps


def kernel(**inputs):
    in_maps = _make_in_maps(**inputs)
    nc = _get_program()
    res = run_bass_kernel_spmd(nc, in_maps, core_ids=list(range(NCORES)))
    shards = [np.asarray(res.results[c]["outT"]) for c in range(NCORES)]
    out = np.concatenate(shards, axis=0)  # (4096, 2048) = (C_out, T)
    return np.ascontiguousarray(out.T)[None].astype(np.float32)  # (1, T, C)
